# revision 4
# baseline (speedup 1.0000x reference)
"""Channel-attention kernel for Trainium2 (8 NeuronCores, data-parallel over batch).

Math: the reference expands x (B,C,T,1) to 8 channels via a 1x1 conv before the
Q@K^T einsum.  Algebraically, with alpha=w1.w2, delta=b1.w2 and
s[b,c]=sum_t x[b,c,t]:

    energy[b,c,e] = alpha*G[b,c,e] + delta*s[b,e] + (terms const along e)
    G[b] = X[b] @ X[b]^T          (X[b] = x[b,:,:,0], shape (C,T))

Terms constant along the e (last) axis cancel in the min-max normalization;
only alpha*G + delta*s_e matters.  This cuts the contraction from T*8 down
to T (the advertised 8x headroom).

v6 design.  v5 used DMA-xbar transposes to build X^T for the Gram; the
xbar TRANSPOSE mode hardware-serializes against ALL passthrough DMA
(known HW deadlock, Tile inserts drain waits), so every one of the 10
transposes forced a full load/store drain: measured strict alternation
of the load/store ring and the transpose ring, ~105us total against a
34us HBM floor.  v6 eliminates the xbar entirely:
  - X^T is built on the TensorEngine: per 128-col chunk k,
    matmul(lhsT=x_bf[:,k*128:...], rhs=I_bf) = chunk^T into PSUM (a
    regular matmul, ~81ns warm, NOT the 275ns transpose-mode path),
    then DVE/ACT copy PSUM->SBUF as bf16 (exact: x*1.0 single-term
    accumulation preserves bf16 values).  4 chunks share one PSUM bank,
    copies split DVE/ACT halves so the copy engines keep up with PE.
  - DMA then carries ONLY the mandatory traffic: 8.2 MB f32 loads
    (sync/HWDGE ring) + 4.1 MB bf16 stores (scalar/HWDGE ring, its own
    FIFO) = 12.3 MB at ~358 GB/s = ~34 us floor, zero mode switches.
  - cast f32->bf16 on DVE (half 0) and ACT (half 1), each with
    accum_out producing its half of the row-sums s -- no reduce pass
  - residual is folded into the attention matmul: lhsT = I + gamma*A,
    so PSUM holds the final output chunk (copy + store, no adds)
  - rank-1 (delta/alpha)*s_e update via a 1-partition matmul appended
    to the Gram PSUM accumulation group
  - x_bf lives in 3 persistent buffers whose [T:TP] pad is zeroed once
    at start; the padded Gram chunk contributes zeros (exact)
"""

import numpy as np
import ml_dtypes
from contextlib import ExitStack

import concourse.bass as bass
import concourse.tile as tile
from concourse import mybir
from concourse.bass_utils import run_bass_kernel_spmd
from concourse.alu_op_type import AluOpType

F32 = mybir.dt.float32
BF16 = mybir.dt.bfloat16
AX = mybir.AxisListType.X

B, C, T = 64, 64, 4000
NCORES = 8
BPC = B // NCORES          # 8 batches per core
PAIRS = BPC // 2           # 4 pairs of 2 batches
ROWS = BPC * C             # 512 rows of (C,T) per core
TP = 4096                  # T padded to a multiple of 128
NKT = TP // 128            # 32 k-tiles
HALF = TP // 2             # cast half boundary (2048 cols)
NCHUNK = 8
CHW = T // NCHUNK          # 500 (fits one PSUM bank in f32)
HALF2 = T // 2             # 2000-col store halves
EPS = 1e-8


def _body(ctx, tc, out_ap, x_ap, idf_ap, alpha, doa, gamma):
    nc = tc.nc

    singles = ctx.enter_context(tc.tile_pool(name="singles", bufs=1))
    xfp = ctx.enter_context(tc.tile_pool(name="xfp", bufs=4))
    xtp = ctx.enter_context(tc.tile_pool(name="xtp", bufs=2))
    obp = ctx.enter_context(tc.tile_pool(name="obp", bufs=2))
    attp = ctx.enter_context(tc.tile_pool(name="attp", bufs=2))
    smalls = ctx.enter_context(tc.tile_pool(name="smalls", bufs=3))
    srp = ctx.enter_context(tc.tile_pool(name="srp", bufs=2))

    ps_t = ctx.enter_context(tc.tile_pool(name="ps_t", bufs=2, space="PSUM"))
    ps_s = ctx.enter_context(tc.tile_pool(name="ps_s", bufs=1, space="PSUM"))
    ps_g = ctx.enter_context(tc.tile_pool(name="ps_g", bufs=1, space="PSUM"))
    ps_o = ctx.enter_context(tc.tile_pool(name="ps_o", bufs=2, space="PSUM"))

    ident_f32 = singles.tile([128, 128], F32)
    nc.sync.dma_start(ident_f32[:], idf_ap)
    ident_bf = singles.tile([128, 128], BF16)
    nc.vector.tensor_copy(ident_bf[:], ident_f32[:])
    ones_row = singles.tile([1, 128], BF16)
    nc.vector.memset(ones_row[:], 1.0)
    # preload the ACT function tables during the ramp
    warm_act = singles.tile([1, 2], F32)
    nc.scalar.activation(
        warm_act[:], ones_row[0:1, 0:2], mybir.ActivationFunctionType.Exp
    )
    # persistent x_bf buffers (rotated p%3); pad zeroed once, here
    xb0 = singles.tile([128, TP], BF16)
    xb1 = singles.tile([128, TP], BF16)
    xb2 = singles.tile([128, TP], BF16)
    xbufs = [xb0, xb1, xb2]
    for xb in xbufs:
        nc.gpsimd.memset(xb[:, T:TP], 0.0)

    st = [{} for _ in range(PAIRS)]

    CSPLIT = 2560  # DVE (2x mode) / ACT cast column split
    QL = TP // 4  # 1024-col quarters (pair 0's loads, for a faster head)

    def stL(p):
        """f32 loads on the sync/HWDGE ring (qSPDynamicHW FIFO carries
        nothing else, so loads stream back-to-back at HBM rate)."""
        v = st[p]
        rows = slice(p * 128, (p + 1) * 128)
        x_f32 = xfp.tile([128, T], F32)
        if p == 0:
            for q in range(4):
                lo, hi = q * QL, min((q + 1) * QL, T)
                nc.sync.dma_start(x_f32[:, lo:hi], x_ap[rows, lo:hi])
        else:
            nc.sync.dma_start(x_f32[:], x_ap[rows, :])
        v["x_f32"] = x_f32
        v["x_bf"] = xbufs[p % 3]

    def stC(p):
        """casts: plain DVE copy (2x_2P fast mode, no accum) + ACT copy for
        the tail; the row-sums come from a GpSimd reduce over x_f32 (the
        Pool engine is otherwise idle and can't touch PSUM anyway)."""
        v = st[p]
        x_f32, x_bf = v["x_f32"], v["x_bf"]
        if p == 0:
            nc.vector.tensor_copy(x_bf[:, 0:QL], x_f32[:, 0:QL])
            nc.vector.tensor_copy(x_bf[:, QL:CSPLIT], x_f32[:, QL:CSPLIT])
        else:
            nc.vector.tensor_copy(x_bf[:, 0:CSPLIT], x_f32[:, 0:CSPLIT])
        nc.scalar.activation(
            x_bf[:, CSPLIT:T], x_f32[:, CSPLIT:T],
            mybir.ActivationFunctionType.Copy,
        )
        # row-sums via a GpSimd pairwise-add tree (Pool is idle; its
        # tensor_reduce can't do the free axis) + a short DVE reduce
        sr1 = srp.tile([128, 2000], F32, tag="sr1")
        nc.gpsimd.tensor_add(sr1[:], x_f32[:, 0:2000], x_f32[:, 2000:4000])
        sr2 = srp.tile([128, 1000], F32, tag="sr2")
        nc.gpsimd.tensor_add(sr2[:], sr1[:, 0:1000], sr1[:, 1000:2000])
        sr3 = srp.tile([128, 500], F32, tag="sr3")
        nc.gpsimd.tensor_add(sr3[:], sr2[:, 0:500], sr2[:, 500:1000])
        s_col = smalls.tile([128, 1], F32, tag="scol")
        nc.vector.tensor_reduce(s_col[:], sr3[:], axis=AX, op=AluOpType.add)
        v["s_col"] = s_col

    def stT(p):
        """X^T via PE identity matmuls: 4 chunk-transposes per PSUM bank,
        then DVE/ACT half-copies to SBUF bf16 (split so the copy engines
        keep pace with the PE stream)."""
        v = st[p]
        x_bf = v["x_bf"]
        xt = xtp.tile([128, TP], BF16)
        for g in range(NKT // 4):
            ps = ps_t.tile([128, 512], F32, tag="t")
            for j in range(4):
                k = 4 * g + j
                nc.tensor.matmul(
                    ps[:, j * 128:(j + 1) * 128],
                    lhsT=x_bf[:, k * 128:(k + 1) * 128],
                    rhs=ident_bf[:],
                    start=True,
                    stop=True,
                )
            base = g * 512
            if g in (0, 3, 6):
                nc.vector.tensor_copy(xt[:, base:base + 512], ps[:])
            else:
                nc.scalar.copy(xt[:, base:base + 512], ps[:])
        v["xt"] = xt

    def stGx(p):
        """Gram matmuls + s-row prep + the aux rank-1 update (PE-dense).
        The s-transpose sits between the halves so the PE never waits on
        work that isn't already due."""
        v = st[p]
        xt = v["xt"]
        psum_g = ps_g.tile([128, 128], F32, tag="g")
        for kt in range(28):
            base = kt * 128
            nc.tensor.matmul(
                psum_g[:],
                lhsT=xt[:, base: base + 128],
                rhs=xt[:, base: base + 128],
                start=(kt == 0),
                stop=False,
            )
        st_ps = ps_s.tile([1, 128], F32, tag="st")
        nc.tensor.transpose(st_ps[:], v["s_col"][:], ident_f32[:])
        rhs_aux = smalls.tile([1, 128], BF16, tag="aux")
        nc.vector.tensor_scalar_mul(rhs_aux[:], st_ps[:], doa)
        for kt in range(28, NKT):
            base = kt * 128
            nc.tensor.matmul(
                psum_g[:],
                lhsT=xt[:, base: base + 128],
                rhs=xt[:, base: base + 128],
                start=False,
                stop=False,
            )
        nc.tensor.matmul(
            psum_g[:],
            lhsT=ones_row[:],
            rhs=rhs_aux[:],
            start=False,
            stop=True,
            skip_group_check=True,
        )
        v["psum_g"] = psum_g

    def stGy(p):
        """energy extraction + min-max softmax -> attention lhsT with the
        residual identity folded in (M = I + gamma*A, block-diagonal)."""
        v = st[p]
        psum_g = v["psum_g"]
        # Diagonal (64,64) blocks, scaled by alpha -> energy (128, 64)
        e_sb = smalls.tile([128, 64], F32, tag="esb")
        nc.vector.tensor_scalar_mul(e_sb[0:64, :], psum_g[0:64, 0:64], alpha)
        nc.vector.tensor_scalar_mul(
            e_sb[64:128, :], psum_g[64:128, 64:128], alpha
        )

        # min-max normalize along free axis, then softmax (normalized values
        # live in [0,1], so no max-subtraction is needed before exp)
        rmax = smalls.tile([128, 1], F32, tag="rmax")
        nc.vector.tensor_reduce(rmax[:], e_sb[:], axis=AX, op=AluOpType.max)
        rmin = smalls.tile([128, 1], F32, tag="rmin")
        nc.vector.tensor_reduce(rmin[:], e_sb[:], axis=AX, op=AluOpType.min)
        den = smalls.tile([128, 1], F32, tag="den")
        nc.vector.tensor_scalar(
            den[:], rmax[:], scalar1=rmin[:], scalar2=EPS,
            op0=AluOpType.subtract, op1=AluOpType.add,
        )
        rden = smalls.tile([128, 1], F32, tag="rden")
        nc.vector.reciprocal(rden[:], den[:])
        nbias = smalls.tile([128, 1], F32, tag="nbias")
        nc.vector.scalar_tensor_tensor(
            nbias[:], in0=rmin[:], scalar=-1.0, in1=rden[:],
            op0=AluOpType.mult, op1=AluOpType.mult,
        )
        ex = smalls.tile([128, 64], F32, tag="ex")
        nc.scalar.activation(
            ex[:], e_sb[:], mybir.ActivationFunctionType.Exp,
            bias=nbias[:], scale=rden[:],
        )
        ssum = smalls.tile([128, 1], F32, tag="ssum")
        nc.vector.tensor_reduce(ssum[:], ex[:], axis=AX, op=AluOpType.add)
        rsum = smalls.tile([128, 1], F32, tag="rsum")
        nc.vector.reciprocal(rsum[:], ssum[:])

        latt0 = attp.tile([128, 128], BF16, tag="latt0")
        nc.gpsimd.memset(latt0[:], 0.0)
        nc.vector.tensor_scalar(
            latt0[0:64, 0:64], ex[0:64, :], scalar1=rsum[0:64], scalar2=gamma,
            op0=AluOpType.mult, op1=AluOpType.mult,
        )
        nc.vector.tensor_scalar(
            latt0[64:128, 64:128], ex[64:128, :], scalar1=rsum[64:128],
            scalar2=gamma, op0=AluOpType.mult, op1=AluOpType.mult,
        )
        latt = attp.tile([128, 128], BF16, tag="latt")
        nc.gpsimd.tensor_add(latt[:], latt0[:], ident_bf[:])
        v["latt"] = latt

    def stA(p):
        """output chunks: PSUM holds the final result (residual folded into
        the matmul).  1024-col chunks span two PSUM banks (each matmul
        stays within one bank); one wide bf16 copy per chunk alternates
        DVE/ACT; stores ride the sync/HWDGE ring behind the loads."""
        v = st[p]
        rows = slice(p * 128, (p + 1) * 128)
        x_bf, latt = v["x_bf"], v["latt"]
        out_sb = obp.tile([128, T], BF16)
        bounds = (0, 1024, 2048, 3072, T)
        for ci in range(4):
            lo, hi = bounds[ci], bounds[ci + 1]
            psum_o = ps_o.tile([128, hi - lo], F32, tag="o")
            nc.tensor.matmul(
                psum_o[:, 0:512], lhsT=latt[:], rhs=x_bf[:, lo:lo + 512],
                start=True, stop=True,
            )
            nc.tensor.matmul(
                psum_o[:, 512:hi - lo], lhsT=latt[:], rhs=x_bf[:, lo + 512:hi],
                start=True, stop=True,
            )
            if ci in (0, 2):
                nc.vector.tensor_copy(out_sb[:, lo:hi], psum_o[:])
            else:
                nc.scalar.copy(out_sb[:, lo:hi], psum_o[:])
            if ci in (1, 3):
                h = ci // 2
                hcols = slice(h * 2048, T if h else 2048)
                nc.sync.dma_start(out_ap[rows, hcols], out_sb[:, hcols])
        v.clear()

    # software-pipelined schedule: PE stream is T0 Gx0 T1 A0 Gx1 T2 A1
    # Gx2 T3 A2 Gx3 A3 -- A(p) sits after T(p+1) so softmax(p) latency
    # hides under the transpose matmuls; casts are issued before Gy so
    # the DVE FIFO never parks a ready cast behind a softmax wait
    sched = [
        (stL, 0), (stC, 0), (stL, 1), (stT, 0),
        (stGx, 0), (stC, 1), (stL, 2), (stGy, 0), (stT, 1),
        (stL, 3), (stA, 0), (stGx, 1), (stC, 2), (stGy, 1), (stT, 2),
        (stA, 1), (stGx, 2), (stC, 3), (stGy, 2), (stT, 3),
        (stA, 2), (stGx, 3), (stGy, 3),
        (stA, 3),
    ]
    for fn, p in sched:
        fn(p)


_MODULE_CACHE = {}


def _build_module(alpha, doa, gamma):
    key = (alpha, doa, gamma)
    if key in _MODULE_CACHE:
        return _MODULE_CACHE[key]
    nc = bass.Bass(
        "TRN2", target_bir_lowering=False, debug=False, num_devices=NCORES
    )
    x_ap = nc.dram_tensor("x", (ROWS, T), F32, kind="ExternalInput").ap()
    idf_ap = nc.dram_tensor("idf", (128, 128), F32, kind="ExternalInput").ap()
    out_ap = nc.dram_tensor("out", (ROWS, T), BF16, kind="ExternalOutput").ap()
    with tile.TileContext(nc) as tc, ExitStack() as ctx:
        _body(ctx, tc, out_ap, x_ap, idf_ap, alpha, doa, gamma)
    if _LEGALIZE_WAITS:
        _split_waits(nc)
    _MODULE_CACHE[key] = nc
    return nc


# The wait-split legalization confuses CoreSim's bookkeeping (hand-built
# NoOps bypass nc.inst_map); tests flip this off for simulation runs.
_LEGALIZE_WAITS = True


def _split_waits(nc):
    """walrus TRN2 codegen allows only ONE sync wait per instruction; when
    Tile emits more (e.g. PSUM slot reuse: previous-writer completion +
    previous-reader), hoist the extras onto same-engine NoOps inserted
    immediately before — the sequencer dispatches in order, so the blocking
    semantics are identical."""
    nid = [0]
    for f in nc.m.functions:
        for block in f.blocks:
            out = []
            for inst in block.instructions:
                si = getattr(inst, "sync_info", None)
                if (
                    si is not None
                    and si.on_wait
                    and len(si.on_wait) > 1
                    and type(inst).__name__ != "InstNoOp"
                ):
                    waits = list(si.on_wait)
                    for w in waits[:-1]:
                        nid[0] += 1
                        out.append(
                            mybir.InstNoOp(
                                name=f"{inst.name}-wsplit{nid[0]}",
                                engine=inst.engine,
                                ins=[],
                                outs=[],
                                sync_info=mybir.SyncInfo(
                                    on_wait=[w], on_update=[]
                                ),
                                text_hint="wait-split",
                                bass_nofuse=True,
                            )
                        )
                    inst.sync_info = mybir.SyncInfo(
                        on_wait=waits[-1:], on_update=list(si.on_update)
                    )
                out.append(inst)
            block.instructions[:] = out


def _prepare(inputs):
    x = np.ascontiguousarray(
        np.asarray(inputs["x"], dtype=np.float32).reshape(B * C, T)
    )
    w1 = np.asarray(inputs["w1"], dtype=np.float64)
    b1 = np.asarray(inputs["b1"], dtype=np.float64)
    w2 = np.asarray(inputs["w2"], dtype=np.float64)
    b2 = np.asarray(inputs["b2"], dtype=np.float64)
    gamma = float(np.asarray(inputs["gamma"]))
    alpha = float(w1 @ w2)
    delta = float(b1 @ w2)
    assert abs(alpha) > 1e-12, "degenerate alpha not supported"
    nc = _build_module(alpha, delta / alpha, gamma)
    ident_f = np.eye(128, dtype=np.float32)
    in_maps = [
        {"x": x[i * ROWS:(i + 1) * ROWS], "idf": ident_f}
        for i in range(NCORES)
    ]
    return nc, in_maps


def kernel(**inputs):
    nc, in_maps = _prepare(inputs)
    res = run_bass_kernel_spmd(nc, in_maps, core_ids=list(range(NCORES)))
    out = np.concatenate([res.results[i]["out"] for i in range(NCORES)], axis=0)
    return out.astype(np.float32).reshape(B, C, T, 1)


# revision 5
# speedup vs baseline: 1.1823x; 1.1823x over previous
"""Channel-attention kernel for Trainium2 (8 NeuronCores, data-parallel over batch).

Math: the reference expands x (B,C,T,1) to 8 channels via a 1x1 conv before the
Q@K^T einsum.  Algebraically, with alpha=w1.w2, delta=b1.w2 and
s[b,c]=sum_t x[b,c,t]:

    energy[b,c,e] = alpha*G[b,c,e] + delta*s[b,e] + (terms const along e)
    G[b] = X[b] @ X[b]^T          (X[b] = x[b,:,:,0], shape (C,T))

Terms constant along the e (last) axis cancel in the min-max normalization;
only alpha*G + delta*s_e matters.  This cuts the contraction from T*8 down
to T (the advertised 8x headroom).

v6 design.  v5 used DMA-xbar transposes to build X^T for the Gram; the
xbar TRANSPOSE mode hardware-serializes against ALL passthrough DMA
(known HW deadlock, Tile inserts drain waits), so every one of the 10
transposes forced a full load/store drain: measured strict alternation
of the load/store ring and the transpose ring, ~105us total against a
34us HBM floor.  v6 eliminates the xbar entirely:
  - X^T is built on the TensorEngine: per 128-col chunk k,
    matmul(lhsT=x_bf[:,k*128:...], rhs=I_bf) = chunk^T into PSUM (a
    regular matmul, ~81ns warm, NOT the 275ns transpose-mode path),
    then DVE/ACT copy PSUM->SBUF as bf16 (exact: x*1.0 single-term
    accumulation preserves bf16 values).  4 chunks share one PSUM bank,
    copies split DVE/ACT halves so the copy engines keep up with PE.
  - DMA then carries ONLY the mandatory traffic: 8.2 MB f32 loads
    (sync/HWDGE ring) + 4.1 MB bf16 stores (scalar/HWDGE ring, its own
    FIFO) = 12.3 MB at ~358 GB/s = ~34 us floor, zero mode switches.
  - cast f32->bf16 on DVE (half 0) and ACT (half 1), each with
    accum_out producing its half of the row-sums s -- no reduce pass
  - residual is folded into the attention matmul: lhsT = I + gamma*A,
    so PSUM holds the final output chunk (copy + store, no adds)
  - rank-1 (delta/alpha)*s_e update via a 1-partition matmul appended
    to the Gram PSUM accumulation group
  - x_bf lives in 3 persistent buffers whose [T:TP] pad is zeroed once
    at start; the padded Gram chunk contributes zeros (exact)
"""

import numpy as np
import ml_dtypes
from contextlib import ExitStack

import concourse.bass as bass
import concourse.tile as tile
from concourse import mybir
from concourse.bass_utils import run_bass_kernel_spmd
from concourse.alu_op_type import AluOpType

F32 = mybir.dt.float32
BF16 = mybir.dt.bfloat16
AX = mybir.AxisListType.X

B, C, T = 64, 64, 4000
NCORES = 8
BPC = B // NCORES          # 8 batches per core
PAIRS = BPC // 2           # 4 pairs of 2 batches
ROWS = BPC * C             # 512 rows of (C,T) per core
TP = 4096                  # T padded to a multiple of 128
NKT = TP // 128            # 32 k-tiles
HALF = TP // 2             # cast half boundary (2048 cols)
NCHUNK = 8
CHW = T // NCHUNK          # 500 (fits one PSUM bank in f32)
HALF2 = T // 2             # 2000-col store halves
EPS = 1e-8


def _body(ctx, tc, out_ap, x_ap, idf_ap, alpha, doa, gamma):
    nc = tc.nc

    singles = ctx.enter_context(tc.tile_pool(name="singles", bufs=1))
    xfp = ctx.enter_context(tc.tile_pool(name="xfp", bufs=4))
    xtp = ctx.enter_context(tc.tile_pool(name="xtp", bufs=2))
    obp = ctx.enter_context(tc.tile_pool(name="obp", bufs=2))
    attp = ctx.enter_context(tc.tile_pool(name="attp", bufs=2))
    smalls = ctx.enter_context(tc.tile_pool(name="smalls", bufs=3))

    ps_t = ctx.enter_context(tc.tile_pool(name="ps_t", bufs=2, space="PSUM"))
    ps_s = ctx.enter_context(tc.tile_pool(name="ps_s", bufs=1, space="PSUM"))
    ps_g = ctx.enter_context(tc.tile_pool(name="ps_g", bufs=1, space="PSUM"))
    ps_o = ctx.enter_context(tc.tile_pool(name="ps_o", bufs=2, space="PSUM"))

    ident_f32 = singles.tile([128, 128], F32)
    nc.sync.dma_start(ident_f32[:], idf_ap)
    ident_bf = singles.tile([128, 128], BF16)
    nc.vector.tensor_copy(ident_bf[:], ident_f32[:])
    ones_row = singles.tile([1, 128], BF16)
    nc.vector.memset(ones_row[:], 1.0)
    # preload the ACT function tables during the ramp
    warm_act = singles.tile([1, 2], F32)
    nc.scalar.activation(
        warm_act[:], ones_row[0:1, 0:2], mybir.ActivationFunctionType.Exp
    )
    # persistent x_bf buffers (rotated p%3); pad zeroed once, here
    xb0 = singles.tile([128, TP], BF16)
    xb1 = singles.tile([128, TP], BF16)
    xb2 = singles.tile([128, TP], BF16)
    xbufs = [xb0, xb1, xb2]
    for xb in xbufs:
        nc.gpsimd.memset(xb[:, T:TP], 0.0)

    st = [{} for _ in range(PAIRS)]

    QL = TP // 4  # 1024-col quarters (pair 0's loads, for a faster head)

    def stL(p):
        """f32 loads on the sync/HWDGE ring (qSPDynamicHW FIFO carries
        nothing else, so loads stream back-to-back at HBM rate)."""
        v = st[p]
        rows = slice(p * 128, (p + 1) * 128)
        x_f32 = xfp.tile([128, T], F32)
        if p == 0:
            for q in range(4):
                lo, hi = q * QL, min((q + 1) * QL, T)
                nc.sync.dma_start(x_f32[:, lo:hi], x_ap[rows, lo:hi])
        else:
            nc.sync.dma_start(x_f32[:], x_ap[rows, :])
        v["x_f32"] = x_f32
        v["x_bf"] = xbufs[p % 3]

    def stC(p):
        """casts with row-sum accumulation: DVE half 0, ACT half 1."""
        v = st[p]
        x_f32, x_bf = v["x_f32"], v["x_bf"]
        s_ab = smalls.tile([128, 3], F32, tag="sab")
        if p == 0:
            nc.vector.tensor_scalar(
                x_bf[:, 0:QL], x_f32[:, 0:QL], scalar1=1.0, scalar2=0.0,
                op0=AluOpType.mult, op1=AluOpType.add, accum_out=s_ab[:, 0:1],
            )
            nc.vector.tensor_scalar(
                x_bf[:, QL:HALF], x_f32[:, QL:HALF], scalar1=1.0, scalar2=0.0,
                op0=AluOpType.mult, op1=AluOpType.add, accum_out=s_ab[:, 2:3],
            )
        else:
            nc.vector.tensor_scalar(
                x_bf[:, 0:HALF], x_f32[:, 0:HALF], scalar1=1.0, scalar2=0.0,
                op0=AluOpType.mult, op1=AluOpType.add, accum_out=s_ab[:, 0:1],
            )
            nc.vector.memset(s_ab[:, 2:3], 0.0)
        nc.scalar.activation(
            x_bf[:, HALF:T], x_f32[:, HALF:T],
            mybir.ActivationFunctionType.Copy, accum_out=s_ab[:, 1:2],
        )
        s_col = smalls.tile([128, 1], F32, tag="scol")
        nc.vector.tensor_reduce(s_col[:], s_ab[:], axis=AX, op=AluOpType.add)
        v["s_col"] = s_col

    def stT(p):
        """X^T via PE identity matmuls: 4 chunk-transposes per PSUM bank,
        then DVE/ACT half-copies to SBUF bf16 (split so the copy engines
        keep pace with the PE stream)."""
        v = st[p]
        x_bf = v["x_bf"]
        xt = xtp.tile([128, TP], BF16)
        for g in range(NKT // 4):
            ps = ps_t.tile([128, 512], F32, tag="t")
            for j in range(4):
                k = 4 * g + j
                nc.tensor.matmul(
                    ps[:, j * 128:(j + 1) * 128],
                    lhsT=x_bf[:, k * 128:(k + 1) * 128],
                    rhs=ident_bf[:],
                    start=True,
                    stop=True,
                )
            base = g * 512
            if g in (0, 3, 6):
                nc.vector.tensor_copy(xt[:, base:base + 512], ps[:])
            else:
                nc.scalar.copy(xt[:, base:base + 512], ps[:])
        v["xt"] = xt

    def stGx(p):
        """Gram matmuls + s-row prep + the aux rank-1 update (PE-dense).
        The s-transpose sits between the halves so the PE never waits on
        work that isn't already due."""
        v = st[p]
        xt = v["xt"]
        psum_g = ps_g.tile([128, 128], F32, tag="g")
        for kt in range(28):
            base = kt * 128
            nc.tensor.matmul(
                psum_g[:],
                lhsT=xt[:, base: base + 128],
                rhs=xt[:, base: base + 128],
                start=(kt == 0),
                stop=False,
            )
        st_ps = ps_s.tile([1, 128], F32, tag="st")
        nc.tensor.transpose(st_ps[:], v["s_col"][:], ident_f32[:])
        rhs_aux = smalls.tile([1, 128], BF16, tag="aux")
        nc.vector.tensor_scalar_mul(rhs_aux[:], st_ps[:], doa)
        for kt in range(28, NKT):
            base = kt * 128
            nc.tensor.matmul(
                psum_g[:],
                lhsT=xt[:, base: base + 128],
                rhs=xt[:, base: base + 128],
                start=False,
                stop=False,
            )
        nc.tensor.matmul(
            psum_g[:],
            lhsT=ones_row[:],
            rhs=rhs_aux[:],
            start=False,
            stop=True,
            skip_group_check=True,
        )
        v["psum_g"] = psum_g

    def stGy(p):
        """energy extraction + min-max softmax -> attention lhsT with the
        residual identity folded in (M = I + gamma*A, block-diagonal)."""
        v = st[p]
        psum_g = v["psum_g"]
        # Diagonal (64,64) blocks, scaled by alpha -> energy (128, 64)
        e_sb = smalls.tile([128, 64], F32, tag="esb")
        nc.vector.tensor_scalar_mul(e_sb[0:64, :], psum_g[0:64, 0:64], alpha)
        nc.vector.tensor_scalar_mul(
            e_sb[64:128, :], psum_g[64:128, 64:128], alpha
        )

        # min-max normalize along free axis, then softmax (normalized values
        # live in [0,1], so no max-subtraction is needed before exp)
        rmax = smalls.tile([128, 1], F32, tag="rmax")
        nc.vector.tensor_reduce(rmax[:], e_sb[:], axis=AX, op=AluOpType.max)
        rmin = smalls.tile([128, 1], F32, tag="rmin")
        nc.vector.tensor_reduce(rmin[:], e_sb[:], axis=AX, op=AluOpType.min)
        den = smalls.tile([128, 1], F32, tag="den")
        nc.vector.tensor_scalar(
            den[:], rmax[:], scalar1=rmin[:], scalar2=EPS,
            op0=AluOpType.subtract, op1=AluOpType.add,
        )
        rden = smalls.tile([128, 1], F32, tag="rden")
        nc.vector.reciprocal(rden[:], den[:])
        nbias = smalls.tile([128, 1], F32, tag="nbias")
        nc.vector.scalar_tensor_tensor(
            nbias[:], in0=rmin[:], scalar=-1.0, in1=rden[:],
            op0=AluOpType.mult, op1=AluOpType.mult,
        )
        ex = smalls.tile([128, 64], F32, tag="ex")
        nc.scalar.activation(
            ex[:], e_sb[:], mybir.ActivationFunctionType.Exp,
            bias=nbias[:], scale=rden[:],
        )
        ssum = smalls.tile([128, 1], F32, tag="ssum")
        nc.vector.tensor_reduce(ssum[:], ex[:], axis=AX, op=AluOpType.add)
        rsum = smalls.tile([128, 1], F32, tag="rsum")
        nc.vector.reciprocal(rsum[:], ssum[:])

        latt0 = attp.tile([128, 128], BF16, tag="latt0")
        nc.vector.memset(latt0[:], 0.0)
        nc.vector.tensor_scalar(
            latt0[0:64, 0:64], ex[0:64, :], scalar1=rsum[0:64], scalar2=gamma,
            op0=AluOpType.mult, op1=AluOpType.mult,
        )
        nc.vector.tensor_scalar(
            latt0[64:128, 64:128], ex[64:128, :], scalar1=rsum[64:128],
            scalar2=gamma, op0=AluOpType.mult, op1=AluOpType.mult,
        )
        latt = attp.tile([128, 128], BF16, tag="latt")
        nc.vector.tensor_add(latt[:], latt0[:], ident_bf[:])
        v["latt"] = latt

    def stA(p):
        """output chunks: PSUM holds the final result (residual folded into
        the matmul).  1024-col chunks span two PSUM banks (each matmul
        stays within one bank); one wide bf16 copy per chunk alternates
        DVE/ACT; stores ride the sync/HWDGE ring behind the loads."""
        v = st[p]
        rows = slice(p * 128, (p + 1) * 128)
        x_bf, latt = v["x_bf"], v["latt"]
        out_sb = obp.tile([128, T], BF16)
        bounds = (0, 1024, 2048, 3072, T)
        for ci in range(4):
            lo, hi = bounds[ci], bounds[ci + 1]
            psum_o = ps_o.tile([128, hi - lo], F32, tag="o")
            nc.tensor.matmul(
                psum_o[:, 0:512], lhsT=latt[:], rhs=x_bf[:, lo:lo + 512],
                start=True, stop=True,
            )
            nc.tensor.matmul(
                psum_o[:, 512:hi - lo], lhsT=latt[:], rhs=x_bf[:, lo + 512:hi],
                start=True, stop=True,
            )
            if ci in (0, 2):
                nc.vector.tensor_copy(out_sb[:, lo:hi], psum_o[:])
            else:
                nc.scalar.copy(out_sb[:, lo:hi], psum_o[:])
            if ci in (1, 3):
                h = ci // 2
                hcols = slice(h * 2048, T if h else 2048)
                nc.sync.dma_start(out_ap[rows, hcols], out_sb[:, hcols])
        v.clear()

    # software-pipelined schedule: PE stream is T0 Gx0 T1 A0 Gx1 T2 A1
    # Gx2 T3 A2 Gx3 A3 -- A(p) sits after T(p+1) so softmax(p) latency
    # hides under the transpose matmuls; casts are issued before Gy so
    # the DVE FIFO never parks a ready cast behind a softmax wait
    sched = [
        (stL, 0), (stC, 0), (stL, 1), (stT, 0),
        (stGx, 0), (stC, 1), (stL, 2), (stGy, 0), (stT, 1),
        (stL, 3), (stA, 0), (stGx, 1), (stC, 2), (stGy, 1), (stT, 2),
        (stA, 1), (stGx, 2), (stC, 3), (stGy, 2), (stT, 3),
        (stA, 2), (stGx, 3), (stGy, 3),
        (stA, 3),
    ]
    for fn, p in sched:
        fn(p)


_MODULE_CACHE = {}


def _build_module(alpha, doa, gamma):
    key = (alpha, doa, gamma)
    if key in _MODULE_CACHE:
        return _MODULE_CACHE[key]
    nc = bass.Bass(
        "TRN2", target_bir_lowering=False, debug=False, num_devices=NCORES
    )
    x_ap = nc.dram_tensor("x", (ROWS, T), F32, kind="ExternalInput").ap()
    idf_ap = nc.dram_tensor("idf", (128, 128), F32, kind="ExternalInput").ap()
    out_ap = nc.dram_tensor("out", (ROWS, T), BF16, kind="ExternalOutput").ap()
    with tile.TileContext(nc) as tc, ExitStack() as ctx:
        _body(ctx, tc, out_ap, x_ap, idf_ap, alpha, doa, gamma)
    if _LEGALIZE_WAITS:
        _split_waits(nc)
    _MODULE_CACHE[key] = nc
    return nc


# The wait-split legalization confuses CoreSim's bookkeeping (hand-built
# NoOps bypass nc.inst_map); tests flip this off for simulation runs.
_LEGALIZE_WAITS = True


def _split_waits(nc):
    """walrus TRN2 codegen allows only ONE sync wait per instruction; when
    Tile emits more (e.g. PSUM slot reuse: previous-writer completion +
    previous-reader), hoist the extras onto same-engine NoOps inserted
    immediately before — the sequencer dispatches in order, so the blocking
    semantics are identical."""
    nid = [0]
    for f in nc.m.functions:
        for block in f.blocks:
            out = []
            for inst in block.instructions:
                si = getattr(inst, "sync_info", None)
                if (
                    si is not None
                    and si.on_wait
                    and len(si.on_wait) > 1
                    and type(inst).__name__ != "InstNoOp"
                ):
                    waits = list(si.on_wait)
                    for w in waits[:-1]:
                        nid[0] += 1
                        out.append(
                            mybir.InstNoOp(
                                name=f"{inst.name}-wsplit{nid[0]}",
                                engine=inst.engine,
                                ins=[],
                                outs=[],
                                sync_info=mybir.SyncInfo(
                                    on_wait=[w], on_update=[]
                                ),
                                text_hint="wait-split",
                                bass_nofuse=True,
                            )
                        )
                    inst.sync_info = mybir.SyncInfo(
                        on_wait=waits[-1:], on_update=list(si.on_update)
                    )
                out.append(inst)
            block.instructions[:] = out


def _prepare(inputs):
    x = np.ascontiguousarray(
        np.asarray(inputs["x"], dtype=np.float32).reshape(B * C, T)
    )
    w1 = np.asarray(inputs["w1"], dtype=np.float64)
    b1 = np.asarray(inputs["b1"], dtype=np.float64)
    w2 = np.asarray(inputs["w2"], dtype=np.float64)
    b2 = np.asarray(inputs["b2"], dtype=np.float64)
    gamma = float(np.asarray(inputs["gamma"]))
    alpha = float(w1 @ w2)
    delta = float(b1 @ w2)
    assert abs(alpha) > 1e-12, "degenerate alpha not supported"
    nc = _build_module(alpha, delta / alpha, gamma)
    ident_f = np.eye(128, dtype=np.float32)
    in_maps = [
        {"x": x[i * ROWS:(i + 1) * ROWS], "idf": ident_f}
        for i in range(NCORES)
    ]
    return nc, in_maps


def kernel(**inputs):
    nc, in_maps = _prepare(inputs)
    res = run_bass_kernel_spmd(nc, in_maps, core_ids=list(range(NCORES)))
    out = np.concatenate([res.results[i]["out"] for i in range(NCORES)], axis=0)
    return out.astype(np.float32).reshape(B, C, T, 1)


# revision 6
# speedup vs baseline: 1.1981x; 1.0134x over previous
"""Channel-attention kernel for Trainium2 (8 NeuronCores, data-parallel over batch).

Math: the reference expands x (B,C,T,1) to 8 channels via a 1x1 conv before the
Q@K^T einsum.  Algebraically, with alpha=w1.w2, delta=b1.w2 and
s[b,c]=sum_t x[b,c,t]:

    energy[b,c,e] = alpha*G[b,c,e] + delta*s[b,e] + (terms const along e)
    G[b] = X[b] @ X[b]^T          (X[b] = x[b,:,:,0], shape (C,T))

Terms constant along the e (last) axis cancel in the min-max normalization;
only alpha*G + delta*s_e matters.  This cuts the contraction from T*8 down
to T (the advertised 8x headroom).

v6 design.  v5 used DMA-xbar transposes to build X^T for the Gram; the
xbar TRANSPOSE mode hardware-serializes against ALL passthrough DMA
(known HW deadlock, Tile inserts drain waits), so every one of the 10
transposes forced a full load/store drain: measured strict alternation
of the load/store ring and the transpose ring, ~105us total against a
34us HBM floor.  v6 eliminates the xbar entirely:
  - X^T is built on the TensorEngine: per 128-col chunk k,
    matmul(lhsT=x_bf[:,k*128:...], rhs=I_bf) = chunk^T into PSUM (a
    regular matmul, ~81ns warm, NOT the 275ns transpose-mode path),
    then DVE/ACT copy PSUM->SBUF as bf16 (exact: x*1.0 single-term
    accumulation preserves bf16 values).  4 chunks share one PSUM bank,
    copies split DVE/ACT halves so the copy engines keep up with PE.
  - DMA then carries ONLY the mandatory traffic: 8.2 MB f32 loads
    (sync/HWDGE ring) + 4.1 MB bf16 stores (scalar/HWDGE ring, its own
    FIFO) = 12.3 MB at ~358 GB/s = ~34 us floor, zero mode switches.
  - cast f32->bf16 on DVE (half 0) and ACT (half 1), each with
    accum_out producing its half of the row-sums s -- no reduce pass
  - residual is folded into the attention matmul: lhsT = I + gamma*A,
    so PSUM holds the final output chunk (copy + store, no adds)
  - rank-1 (delta/alpha)*s_e update via a 1-partition matmul appended
    to the Gram PSUM accumulation group
  - x_bf lives in 3 persistent buffers whose [T:TP] pad is zeroed once
    at start; the padded Gram chunk contributes zeros (exact)
"""

import numpy as np
import ml_dtypes
from contextlib import ExitStack

import concourse.bass as bass
import concourse.tile as tile
from concourse import mybir
from concourse.bass_utils import run_bass_kernel_spmd
from concourse.alu_op_type import AluOpType

F32 = mybir.dt.float32
BF16 = mybir.dt.bfloat16
AX = mybir.AxisListType.X

B, C, T = 64, 64, 4000
NCORES = 8
BPC = B // NCORES          # 8 batches per core
PAIRS = BPC // 2           # 4 pairs of 2 batches
ROWS = BPC * C             # 512 rows of (C,T) per core
TP = 4096                  # T padded to a multiple of 128
NKT = TP // 128            # 32 k-tiles
HALF = TP // 2             # cast half boundary (2048 cols)
NCHUNK = 8
CHW = T // NCHUNK          # 500 (fits one PSUM bank in f32)
HALF2 = T // 2             # 2000-col store halves
EPS = 1e-8


def _body(ctx, tc, out_ap, x_ap, idf_ap, alpha, doa, gamma):
    nc = tc.nc

    singles = ctx.enter_context(tc.tile_pool(name="singles", bufs=1))
    xfp = ctx.enter_context(tc.tile_pool(name="xfp", bufs=4))
    xtp = ctx.enter_context(tc.tile_pool(name="xtp", bufs=2))
    obp = ctx.enter_context(tc.tile_pool(name="obp", bufs=2))
    smalls = ctx.enter_context(tc.tile_pool(name="smalls", bufs=3))

    # PSUM budget (8 banks): ps_t 2x(128,1024)=4, ps_g 1, ps_s 1, ps_o 2
    ps_t = ctx.enter_context(tc.tile_pool(name="ps_t", bufs=2, space="PSUM"))
    ps_s = ctx.enter_context(tc.tile_pool(name="ps_s", bufs=1, space="PSUM"))
    ps_g = ctx.enter_context(tc.tile_pool(name="ps_g", bufs=1, space="PSUM"))
    ps_o = ctx.enter_context(tc.tile_pool(name="ps_o", bufs=2, space="PSUM"))

    ident_f32 = singles.tile([128, 128], F32)
    nc.sync.dma_start(ident_f32[:], idf_ap)
    ident_bf = singles.tile([128, 128], BF16)
    nc.vector.tensor_copy(ident_bf[:], ident_f32[:])
    ones_row = singles.tile([1, 128], BF16)
    nc.vector.memset(ones_row[:], 1.0)
    # preload the ACT function tables during the ramp
    warm_act = singles.tile([1, 2], F32)
    nc.scalar.activation(
        warm_act[:], ones_row[0:1, 0:2], mybir.ActivationFunctionType.Exp
    )
    # persistent x_bf buffers (rotated p%3); pad zeroed once, here
    xb0 = singles.tile([128, TP], BF16)
    xb1 = singles.tile([128, TP], BF16)
    xb2 = singles.tile([128, TP], BF16)
    xbufs = [xb0, xb1, xb2]
    for xb in xbufs:
        nc.gpsimd.memset(xb[:, T:TP], 0.0)
    # persistent latt bufs (rotated p%2): off-diagonal blocks stay zero
    # forever, so each pair only writes its two diagonal 64-blocks
    la0 = singles.tile([128, 128], BF16)
    la1 = singles.tile([128, 128], BF16)
    lattbufs = [la0, la1]
    for la in lattbufs:
        nc.gpsimd.memset(la[:], 0.0)

    st = [{} for _ in range(PAIRS)]

    QL = TP // 4  # 1024-col quarters (pair 0's loads, for a faster head)
    CSP = 1824    # cast split: DVE (1.11 ns/col w/ accum) vs ACT (0.91)

    def stL(p):
        """f32 loads on the sync/HWDGE ring (qSPDynamicHW FIFO carries
        nothing else, so loads stream back-to-back at HBM rate)."""
        v = st[p]
        rows = slice(p * 128, (p + 1) * 128)
        x_f32 = xfp.tile([128, T], F32)
        if p == 0:
            for q in range(4):
                lo, hi = q * QL, min((q + 1) * QL, T)
                nc.sync.dma_start(x_f32[:, lo:hi], x_ap[rows, lo:hi])
        else:
            nc.sync.dma_start(x_f32[:], x_ap[rows, :])
        v["x_f32"] = x_f32
        v["x_bf"] = xbufs[p % 3]

    def stC(p):
        """casts with row-sum accumulation: DVE half 0, ACT half 1."""
        v = st[p]
        x_f32, x_bf = v["x_f32"], v["x_bf"]
        s_ab = smalls.tile([128, 3], F32, tag="sab")
        if p == 0:
            nc.vector.tensor_scalar(
                x_bf[:, 0:QL], x_f32[:, 0:QL], scalar1=1.0, scalar2=0.0,
                op0=AluOpType.mult, op1=AluOpType.add, accum_out=s_ab[:, 0:1],
            )
            nc.vector.tensor_scalar(
                x_bf[:, QL:CSP], x_f32[:, QL:CSP], scalar1=1.0, scalar2=0.0,
                op0=AluOpType.mult, op1=AluOpType.add, accum_out=s_ab[:, 2:3],
            )
        else:
            nc.vector.tensor_scalar(
                x_bf[:, 0:CSP], x_f32[:, 0:CSP], scalar1=1.0, scalar2=0.0,
                op0=AluOpType.mult, op1=AluOpType.add, accum_out=s_ab[:, 0:1],
            )
            nc.vector.memset(s_ab[:, 2:3], 0.0)
        nc.scalar.activation(
            x_bf[:, CSP:T], x_f32[:, CSP:T],
            mybir.ActivationFunctionType.Copy, accum_out=s_ab[:, 1:2],
        )
        s_col = smalls.tile([128, 1], F32, tag="scol")
        nc.vector.tensor_reduce(s_col[:], s_ab[:], axis=AX, op=AluOpType.add)
        v["s_col"] = s_col

    def stT(p):
        """X^T via PE identity matmuls: 4 chunk-transposes per PSUM bank,
        then DVE/ACT half-copies to SBUF bf16 (split so the copy engines
        keep pace with the PE stream)."""
        v = st[p]
        x_bf = v["x_bf"]
        xt = xtp.tile([128, TP], BF16)
        for g in range(NKT // 8):
            ps = ps_t.tile([128, 1024], F32, tag="t")
            for j in range(8):
                k = 8 * g + j
                nc.tensor.matmul(
                    ps[:, j * 128:(j + 1) * 128],
                    lhsT=x_bf[:, k * 128:(k + 1) * 128],
                    rhs=ident_bf[:],
                    start=True,
                    stop=True,
                )
            base = g * 1024
            if g % 2 == 0:
                nc.vector.tensor_copy(xt[:, base:base + 1024], ps[:])
            else:
                nc.scalar.copy(xt[:, base:base + 1024], ps[:])
        v["xt"] = xt

    def stGx(p):
        """Gram matmuls + s-row prep + the aux rank-1 update (PE-dense).
        The s-transpose sits between the halves so the PE never waits on
        work that isn't already due."""
        v = st[p]
        xt = v["xt"]
        psum_g = ps_g.tile([128, 128], F32, tag="g")
        for kt in range(28):
            base = kt * 128
            nc.tensor.matmul(
                psum_g[:],
                lhsT=xt[:, base: base + 128],
                rhs=xt[:, base: base + 128],
                start=(kt == 0),
                stop=False,
            )
        st_ps = ps_s.tile([1, 128], F32, tag="st")
        nc.tensor.transpose(st_ps[:], v["s_col"][:], ident_f32[:])
        rhs_aux = smalls.tile([1, 128], BF16, tag="aux")
        nc.vector.tensor_scalar_mul(rhs_aux[:], st_ps[:], doa)
        for kt in range(28, NKT):
            base = kt * 128
            nc.tensor.matmul(
                psum_g[:],
                lhsT=xt[:, base: base + 128],
                rhs=xt[:, base: base + 128],
                start=False,
                stop=False,
            )
        nc.tensor.matmul(
            psum_g[:],
            lhsT=ones_row[:],
            rhs=rhs_aux[:],
            start=False,
            stop=True,
            skip_group_check=True,
        )
        v["psum_g"] = psum_g

    def stGy(p):
        """energy extraction + min-max softmax -> attention lhsT with the
        residual identity folded in (M = I + gamma*A, block-diagonal)."""
        v = st[p]
        psum_g = v["psum_g"]
        # Diagonal (64,64) blocks, scaled by alpha -> energy (128, 64)
        e_sb = smalls.tile([128, 64], F32, tag="esb")
        nc.vector.tensor_scalar_mul(e_sb[0:64, :], psum_g[0:64, 0:64], alpha)
        nc.vector.tensor_scalar_mul(
            e_sb[64:128, :], psum_g[64:128, 64:128], alpha
        )

        # min-max normalize along free axis, then softmax (normalized values
        # live in [0,1], so no max-subtraction is needed before exp)
        rmax = smalls.tile([128, 1], F32, tag="rmax")
        nc.vector.tensor_reduce(rmax[:], e_sb[:], axis=AX, op=AluOpType.max)
        rmin = smalls.tile([128, 1], F32, tag="rmin")
        nc.vector.tensor_reduce(rmin[:], e_sb[:], axis=AX, op=AluOpType.min)
        den = smalls.tile([128, 1], F32, tag="den")
        nc.vector.tensor_scalar(
            den[:], rmax[:], scalar1=rmin[:], scalar2=EPS,
            op0=AluOpType.subtract, op1=AluOpType.add,
        )
        rden = smalls.tile([128, 1], F32, tag="rden")
        nc.vector.reciprocal(rden[:], den[:])
        nbias = smalls.tile([128, 1], F32, tag="nbias")
        nc.vector.scalar_tensor_tensor(
            nbias[:], in0=rmin[:], scalar=-1.0, in1=rden[:],
            op0=AluOpType.mult, op1=AluOpType.mult,
        )
        ex = smalls.tile([128, 64], F32, tag="ex")
        nc.scalar.activation(
            ex[:], e_sb[:], mybir.ActivationFunctionType.Exp,
            bias=nbias[:], scale=rden[:],
        )
        ssum = smalls.tile([128, 1], F32, tag="ssum")
        nc.vector.tensor_reduce(ssum[:], ex[:], axis=AX, op=AluOpType.add)
        rsum = smalls.tile([128, 1], F32, tag="rsum")
        nc.vector.reciprocal(rsum[:], ssum[:])
        rsg = smalls.tile([128, 1], F32, tag="rsg")
        nc.vector.tensor_scalar_mul(rsg[:], rsum[:], gamma)

        # write gamma*A + I directly into the persistent latt's diagonal
        # 64-blocks (off-diagonal blocks are zero from the one-time memset)
        latt = lattbufs[p % 2]
        nc.vector.scalar_tensor_tensor(
            latt[0:64, 0:64], in0=ex[0:64, :], scalar=rsg[0:64],
            in1=ident_bf[0:64, 0:64], op0=AluOpType.mult, op1=AluOpType.add,
        )
        nc.vector.scalar_tensor_tensor(
            latt[64:128, 64:128], in0=ex[64:128, :], scalar=rsg[64:128],
            in1=ident_bf[64:128, 64:128], op0=AluOpType.mult,
            op1=AluOpType.add,
        )
        v["latt"] = latt

    def stA(p):
        """output chunks: PSUM holds the final result (residual folded into
        the matmul).  1024-col chunks span two PSUM banks (each matmul
        stays within one bank); one wide bf16 copy per chunk alternates
        DVE/ACT; stores ride the sync/HWDGE ring behind the loads."""
        v = st[p]
        rows = slice(p * 128, (p + 1) * 128)
        x_bf, latt = v["x_bf"], v["latt"]
        out_sb = obp.tile([128, T], BF16)
        for ci in range(8):
            lo, hi = ci * 512, min((ci + 1) * 512, T)
            psum_o = ps_o.tile([128, hi - lo], F32, tag="o")
            nc.tensor.matmul(
                psum_o[:], lhsT=latt[:], rhs=x_bf[:, lo:hi],
                start=True, stop=True,
            )
            if ci in (0, 3, 6):
                nc.vector.tensor_copy(out_sb[:, lo:hi], psum_o[:])
            else:
                nc.scalar.copy(out_sb[:, lo:hi], psum_o[:])
            if ci in (3, 7):
                h = ci // 4
                hcols = slice(h * 2048, T if h else 2048)
                nc.sync.dma_start(out_ap[rows, hcols], out_sb[:, hcols])
        v.clear()

    # software-pipelined schedule: PE stream is T0 Gx0 T1 A0 Gx1 T2 A1
    # Gx2 T3 A2 Gx3 A3 -- A(p) sits after T(p+1) so softmax(p) latency
    # hides under the transpose matmuls; casts are issued before Gy so
    # the DVE FIFO never parks a ready cast behind a softmax wait
    sched = [
        (stL, 0), (stC, 0), (stL, 1), (stT, 0),
        (stGx, 0), (stC, 1), (stL, 2), (stGy, 0), (stT, 1),
        (stL, 3), (stA, 0), (stGx, 1), (stC, 2), (stGy, 1), (stT, 2),
        (stA, 1), (stGx, 2), (stC, 3), (stGy, 2), (stT, 3),
        (stA, 2), (stGx, 3), (stGy, 3),
        (stA, 3),
    ]
    for fn, p in sched:
        fn(p)


_MODULE_CACHE = {}


def _build_module(alpha, doa, gamma):
    key = (alpha, doa, gamma)
    if key in _MODULE_CACHE:
        return _MODULE_CACHE[key]
    nc = bass.Bass(
        "TRN2", target_bir_lowering=False, debug=False, num_devices=NCORES
    )
    x_ap = nc.dram_tensor("x", (ROWS, T), F32, kind="ExternalInput").ap()
    idf_ap = nc.dram_tensor("idf", (128, 128), F32, kind="ExternalInput").ap()
    out_ap = nc.dram_tensor("out", (ROWS, T), BF16, kind="ExternalOutput").ap()
    with tile.TileContext(nc) as tc, ExitStack() as ctx:
        _body(ctx, tc, out_ap, x_ap, idf_ap, alpha, doa, gamma)
    if _LEGALIZE_WAITS:
        _split_waits(nc)
    _MODULE_CACHE[key] = nc
    return nc


# The wait-split legalization confuses CoreSim's bookkeeping (hand-built
# NoOps bypass nc.inst_map); tests flip this off for simulation runs.
_LEGALIZE_WAITS = True


def _split_waits(nc):
    """walrus TRN2 codegen allows only ONE sync wait per instruction; when
    Tile emits more (e.g. PSUM slot reuse: previous-writer completion +
    previous-reader), hoist the extras onto same-engine NoOps inserted
    immediately before — the sequencer dispatches in order, so the blocking
    semantics are identical."""
    nid = [0]
    for f in nc.m.functions:
        for block in f.blocks:
            out = []
            for inst in block.instructions:
                si = getattr(inst, "sync_info", None)
                if (
                    si is not None
                    and si.on_wait
                    and len(si.on_wait) > 1
                    and type(inst).__name__ != "InstNoOp"
                ):
                    waits = list(si.on_wait)
                    for w in waits[:-1]:
                        nid[0] += 1
                        out.append(
                            mybir.InstNoOp(
                                name=f"{inst.name}-wsplit{nid[0]}",
                                engine=inst.engine,
                                ins=[],
                                outs=[],
                                sync_info=mybir.SyncInfo(
                                    on_wait=[w], on_update=[]
                                ),
                                text_hint="wait-split",
                                bass_nofuse=True,
                            )
                        )
                    inst.sync_info = mybir.SyncInfo(
                        on_wait=waits[-1:], on_update=list(si.on_update)
                    )
                out.append(inst)
            block.instructions[:] = out


def _prepare(inputs):
    x = np.ascontiguousarray(
        np.asarray(inputs["x"], dtype=np.float32).reshape(B * C, T)
    )
    w1 = np.asarray(inputs["w1"], dtype=np.float64)
    b1 = np.asarray(inputs["b1"], dtype=np.float64)
    w2 = np.asarray(inputs["w2"], dtype=np.float64)
    b2 = np.asarray(inputs["b2"], dtype=np.float64)
    gamma = float(np.asarray(inputs["gamma"]))
    alpha = float(w1 @ w2)
    delta = float(b1 @ w2)
    assert abs(alpha) > 1e-12, "degenerate alpha not supported"
    nc = _build_module(alpha, delta / alpha, gamma)
    ident_f = np.eye(128, dtype=np.float32)
    in_maps = [
        {"x": x[i * ROWS:(i + 1) * ROWS], "idf": ident_f}
        for i in range(NCORES)
    ]
    return nc, in_maps


def kernel(**inputs):
    nc, in_maps = _prepare(inputs)
    res = run_bass_kernel_spmd(nc, in_maps, core_ids=list(range(NCORES)))
    out = np.concatenate([res.results[i]["out"] for i in range(NCORES)], axis=0)
    return out.astype(np.float32).reshape(B, C, T, 1)
